# revision 1
# baseline (speedup 1.0000x reference)
"""CoNHD GD-layer Trainium2 kernel (8-core SPMD, Bass/Tile).

Math (see module docstring of the reference): two independent set-attention
stacks over fixed-size mailbox groups (v-side: N=2048 nodes x DV=32, e-side:
M=4096 hyperedges x DE=16), followed by a 4*D -> D update linear applied in
two eid orders.

Device strategy:
  - Shard rows (E=65536) across 8 cores: core c owns v-rows and e-rows
    [c*8192, (c+1)*8192).  Group attention never crosses that boundary.
  - Transposed activation layout on chip: SBUF tiles are [feat, rows].
  - All matmuls in fp32r (full PE rate at moving-dim 256, ~1e-4 rel err).
  - Block-diagonal group masking is folded into the score matmul as a
    rank-G accumulation (Gk^T @ Gq, scaled 16C), removed by exp(x/16 - C).
  - Softmax denominator comes from a ones-column appended to V (65-stride
    layout); normalization uses a K=1 ones-matmul broadcast of 1/denom.
  - The update linear is decomposed by column blocks of upd_W:
      A  = co_in@W1 + co_v@W2 + co_0@W4 + b   ('in' eid order, on device)
      P3 = co_e@W3                            ('con' eid order, on device)
      out_in  = A + P3[inv_perm]              (host add + gather)
      out_con = out_in[perm]                  (host gather)

kernel(**inputs) takes the full unsharded inputs and returns [2, E, D] f32.
"""
import sys

if "/opt/trn_rl_repo" not in sys.path:
    sys.path.insert(0, "/opt/trn_rl_repo")

from contextlib import ExitStack

import numpy as np

import concourse.mybir as mybir
import concourse.tile as tile
from concourse import bacc
from concourse.bass_utils import run_bass_kernel_spmd

F32 = mybir.dt.float32
F32R = mybir.dt.float32r
AF = mybir.ActivationFunctionType

N, DV, M, DE, E = 2048, 32, 4096, 16, 65536
D, WD, L, H = 256, 64, 2, 4
NCORES = 8
MASK_C = 30.0


def _sab_tile(nc, pools, Xt, W, Bcol, bvbc, Gk, Gq, ones1, negc):
    """One SAB layer on one 256-row tile (Xt = [feat,rows] tile pair)."""
    sb, psum_mm, psum_S, psum_O = pools

    Qt = [sb.tile([128, 256], F32R, tag="Qt", name="Qt") for _ in range(2)]
    Kt = [sb.tile([128, 256], F32R, tag="Kt", name="Kt") for _ in range(2)]
    for fb in range(2):
        psQ = psum_mm.tile([128, 256], F32, tag="mm", name="psQ")
        for kb in range(2):
            nc.tensor.matmul(psQ[:], W["q"][kb][:, fb * 128:(fb + 1) * 128],
                             Xt[kb][:], start=(kb == 0), stop=(kb == 1))
        nc.vector.tensor_scalar_add(Qt[fb][:], psQ[:], Bcol[:, 0 * 2 + fb:0 * 2 + fb + 1])
        psK = psum_mm.tile([128, 256], F32, tag="mm", name="psK")
        for kb in range(2):
            nc.tensor.matmul(psK[:], W["k"][kb][:, fb * 128:(fb + 1) * 128],
                             Xt[kb][:], start=(kb == 0), stop=(kb == 1))
        nc.vector.tensor_scalar_add(Kt[fb][:], psK[:], Bcol[:, 1 * 2 + fb:1 * 2 + fb + 1])

    # V in row-major 65-stride layout; col h*65+64 holds ones -> softmax denom
    V65 = []
    for rb in range(2):
        psV = psum_mm.tile([128, 256], F32, tag="mm", name="psV")
        for kb in range(2):
            nc.tensor.matmul(psV[:], Xt[kb][:, rb * 128:(rb + 1) * 128],
                             W["v"][kb][:], start=(kb == 0), stop=(kb == 1))
        v65 = sb.tile([128, 4 * 65], F32R, tag="V65", name="V65")
        for h in range(H):
            nc.vector.tensor_add(v65[:, h * 65:h * 65 + 64],
                                 psV[:, h * 64:(h + 1) * 64],
                                 bvbc[:, h * 64:(h + 1) * 64])
        nc.vector.tensor_copy(v65[:, 64::65], ones1[:, 0:4])
        V65.append(v65)

    # attention per head-pair (2 psO banks live at a time)
    Ot = [sb.tile([128, 256], F32R, tag="Ot", name="Ot") for _ in range(2)]
    for pair in range(2):
        psO, recips = [], []
        for hh in range(2):
            h = pair * 2 + hh
            off = hh * 64
            Qht = Qt[pair][off:off + 64, :]
            Kht = Kt[pair][off:off + 64, :]
            eS = []
            for b in range(2):
                psS = psum_S.tile([128, 256], F32, tag="psS", name="psS")
                nc.tensor.matmul(psS[:], Kht[:, b * 128:(b + 1) * 128], Qht,
                                 start=True, stop=False)
                nc.tensor.matmul(psS[:], Gk[:], Gq[b][:], start=False, stop=True)
                e = sb.tile([128, 256], F32R, tag="eS", name="eS")
                nc.scalar.activation(e[:], psS[:], AF.Exp, bias=negc[:], scale=1.0 / 16.0)
                eS.append(e)
            pO = psum_O.tile([65, 256], F32, tag="psO", name="psO")
            for b in range(2):
                nc.tensor.matmul(pO[:], V65[b][:, h * 65:h * 65 + 65], eS[b][:],
                                 start=(b == 0), stop=(b == 1))
            rec = sb.tile([1, 256], F32R, tag="recipH", name="recipH")
            nc.vector.reciprocal(rec[:], pO[64:65, :])
            psO.append(pO)
            recips.append(rec)
        RB = sb.tile([128, 256], F32, tag="RB", name="RB")
        for hh in range(2):
            psRB = psum_S.tile([64, 256], F32, tag="psS", name="psRB")
            nc.tensor.matmul(psRB[:], ones1[0:1, 0:64], recips[hh][:],
                             start=True, stop=True)
            nc.scalar.copy(RB[hh * 64:(hh + 1) * 64, :], psRB[:])
        for hh in range(2):
            off = hh * 64
            nc.vector.tensor_mul(Ot[pair][off:off + 64, :], psO[hh][0:64, :],
                                 RB[off:off + 64, :])
            nc.vector.tensor_add(Ot[pair][off:off + 64, :], Ot[pair][off:off + 64, :],
                                 Qt[pair][off:off + 64, :])

    # Z = O + relu(O @ Wo + bo)
    Zt = [sb.tile([128, 256], F32R, tag="Zt", name="Zt") for _ in range(2)]
    for fb in range(2):
        psR = psum_mm.tile([128, 256], F32, tag="mm", name="psR")
        for kb in range(2):
            nc.tensor.matmul(psR[:], W["o"][kb][:, fb * 128:(fb + 1) * 128],
                             Ot[kb][:], start=(kb == 0), stop=(kb == 1))
        Rt = sb.tile([128, 256], F32, tag="Rt", name="Rt")
        nc.scalar.activation(Rt[:], psR[:], AF.Relu,
                             bias=Bcol[:, 3 * 2 + fb:3 * 2 + fb + 1])
        nc.vector.tensor_add(Zt[fb][:], Ot[fb][:], Rt[:])
    return Zt


def _load_side_consts(nc, const, tag, W_d, Bcol_d, bvbc_d, Gk_d, Gq_d, G):
    Ws, Bcols, bvbcs = [], [], []
    for l in range(L):
        Wl = {}
        for pi, p in enumerate(["q", "k", "v", "o"]):
            Wl[p] = []
            for kb in range(2):
                t = const.tile([128, 256], F32R, tag=f"{tag}W{l}{p}{kb}",
                               name=f"{tag}W{l}{p}{kb}")
                nc.sync.dma_start(t[:], W_d[l, pi, kb * 128:(kb + 1) * 128, :])
                Wl[p].append(t)
        Ws.append(Wl)
        bc = const.tile([128, 8], F32, tag=f"{tag}Bcol{l}", name=f"{tag}Bcol{l}")
        nc.sync.dma_start(bc[:], Bcol_d[l])
        Bcols.append(bc)
        bv = const.tile([128, 256], F32, tag=f"{tag}bvbc{l}", name=f"{tag}bvbc{l}")
        nc.sync.dma_start(bv[:], bvbc_d[l])
        bvbcs.append(bv)
    Gk = const.tile([G, 128], F32R, tag=f"{tag}Gk", name=f"{tag}Gk")
    nc.sync.dma_start(Gk[:], Gk_d)
    Gq = []
    for b in range(2):
        g = const.tile([G, 256], F32R, tag=f"{tag}Gq{b}", name=f"{tag}Gq{b}")
        nc.sync.dma_start(g[:], Gq_d[b])
        Gq.append(g)
    return Ws, Bcols, bvbcs, Gk, Gq


def build_program(R):
    """Build the per-core SPMD program; R = rows per core (multiple of 256)."""
    NT = R // 256
    nc = bacc.Bacc("TRN2", target_bir_lowering=False, debug=False)

    dram = {}

    def din(name, shape, dt=F32R):
        dram[name] = nc.dram_tensor(name, shape, dt, kind="ExternalInput").ap()
        return dram[name]

    xvt_d = din("xvt", [D, R])
    wvt_d = din("wvt", [WD, R])
    xet_d = din("xet", [D, R])
    wet_d = din("wet", [WD, R])
    x0t_d = din("x0t", [D, R])
    peW_v_d = din("peW_v", [WD, D])
    peW_e_d = din("peW_e", [WD, D])
    peb_v_d = din("peb_v", [D], F32)
    peb_e_d = din("peb_e", [D], F32)
    Wv_d = din("W_v", [L, 4, D, D])
    We_d = din("W_e", [L, 4, D, D])
    Bcol_v_d = din("Bcol_v", [L, 128, 8], F32)
    Bcol_e_d = din("Bcol_e", [L, 128, 8], F32)
    bvbc_v_d = din("bvbc_v", [L, 128, D], F32)
    bvbc_e_d = din("bvbc_e", [L, 128, D], F32)
    Wupd_d = din("W_upd", [4, D, D])
    updb_d = din("updb_bc", [128, D], F32)
    Gk_v_d = din("Gk_v", [4, 128])
    Gq_v_d = din("Gq_v", [2, 4, 256])
    Gk_e_d = din("Gk_e", [8, 128])
    Gq_e_d = din("Gq_e", [2, 8, 256])
    ones1_d = din("ones1", [128, 128])

    A_d = nc.dram_tensor("A", [R, D], F32, kind="ExternalOutput").ap()
    P3_d = nc.dram_tensor("P3", [R, D], F32, kind="ExternalOutput").ap()

    with tile.TileContext(nc) as tc, ExitStack() as es, \
            nc.allow_low_precision(reason="fp32r matmul pipeline, fp32 accum in PSUM"):
        const = es.enter_context(tc.tile_pool(name="const", bufs=1))
        sb = es.enter_context(tc.tile_pool(name="sb", bufs=4))
        inp = es.enter_context(tc.tile_pool(name="inp", bufs=4))
        outp = es.enter_context(tc.tile_pool(name="outp", bufs=4))
        psum_mm = es.enter_context(tc.tile_pool(name="psmm", bufs=3, space="PSUM"))
        psum_S = es.enter_context(tc.tile_pool(name="psS", bufs=3, space="PSUM"))
        psum_O = es.enter_context(tc.tile_pool(name="psO", bufs=2, space="PSUM"))
        pools = (sb, psum_mm, psum_S, psum_O)

        negc = const.tile([128, 1], F32, tag="negc", name="negc")
        nc.vector.memset(negc[:], -MASK_C)
        ones1 = const.tile([128, 128], F32R, tag="ones1", name="ones1")
        nc.sync.dma_start(ones1[:], ones1_d)

        peW = {}
        peb = {}
        for s, peW_d, peb_d in (("v", peW_v_d, peb_v_d), ("e", peW_e_d, peb_e_d)):
            t = const.tile([WD, D], F32R, tag=f"peW_{s}", name=f"peW_{s}")
            nc.sync.dma_start(t[:], peW_d)
            peW[s] = t
            b = const.tile([128, 2], F32, tag=f"peb_{s}", name=f"peb_{s}")
            for fb in range(2):
                nc.sync.dma_start(b[:, fb:fb + 1],
                                  peb_d[fb * 128:(fb + 1) * 128].unsqueeze(-1))
            peb[s] = b

        side_consts = {
            "v": _load_side_consts(nc, const, "v", Wv_d, Bcol_v_d, bvbc_v_d,
                                   Gk_v_d, Gq_v_d, 4),
            "e": _load_side_consts(nc, const, "e", We_d, Bcol_e_d, bvbc_e_d,
                                   Gk_e_d, Gq_e_d, 8),
        }

        Wupd = []
        for j in range(4):
            Wupd.append([])
            for kb in range(2):
                t = const.tile([128, 256], F32R, tag=f"Wupd{j}{kb}", name=f"Wupd{j}{kb}")
                nc.sync.dma_start(t[:], Wupd_d[j, kb * 128:(kb + 1) * 128, :])
                Wupd[j].append(t)
        updb = const.tile([128, 256], F32, tag="updb", name="updb")
        nc.sync.dma_start(updb[:], updb_d)

        for side in ("v", "e"):
            Ws, Bcols, bvbcs, Gk, Gq = side_consts[side]
            xt_d, wt_d = (xvt_d, wvt_d) if side == "v" else (xet_d, wet_d)
            for t in range(NT):
                cs = slice(t * 256, (t + 1) * 256)
                xt = [inp.tile([128, 256], F32R, tag=f"xt{side}", name="xt")
                      for _ in range(2)]
                for fb in range(2):
                    nc.sync.dma_start(xt[fb][:], xt_d[fb * 128:(fb + 1) * 128, cs])
                wt = inp.tile([WD, 256], F32R, tag=f"wt{side}", name="wt")
                nc.sync.dma_start(wt[:], wt_d[:, cs])

                # mailbox: Xt = xt + peW^T wt + peb
                Xt = [sb.tile([128, 256], F32R, tag="Xt", name="Xt") for _ in range(2)]
                for fb in range(2):
                    psP = psum_mm.tile([128, 256], F32, tag="mm", name="psP")
                    nc.tensor.matmul(psP[:], peW[side][:, fb * 128:(fb + 1) * 128],
                                     wt[:], start=True, stop=True)
                    nc.vector.scalar_tensor_tensor(
                        Xt[fb][:], psP[:], peb[side][:, fb:fb + 1], xt[fb][:],
                        mybir.AluOpType.add, mybir.AluOpType.add)

                for l in range(L):
                    Xt = _sab_tile(nc, pools, Xt, Ws[l], Bcols[l], bvbcs[l],
                                   Gk, Gq, ones1, negc)

                if side == "v":
                    x0 = [inp.tile([128, 256], F32R, tag="x0", name="x0")
                          for _ in range(2)]
                    for fb in range(2):
                        nc.sync.dma_start(x0[fb][:], x0t_d[fb * 128:(fb + 1) * 128, cs])
                    for rb in range(2):
                        rs = slice(rb * 128, (rb + 1) * 128)
                        psA = psum_mm.tile([128, 256], F32, tag="mm", name="psA")
                        first = True
                        for src, j in ((xt, 0), (Xt, 1), (x0, 3)):
                            for kb in range(2):
                                nc.tensor.matmul(psA[:], src[kb][:, rs], Wupd[j][kb][:],
                                                 start=first,
                                                 stop=(src is x0 and kb == 1))
                                first = False
                        Ao = outp.tile([128, 256], F32, tag="Aout", name="Aout")
                        nc.vector.tensor_add(Ao[:], psA[:], updb[:])
                        nc.sync.dma_start(A_d[t * 256 + rb * 128:t * 256 + (rb + 1) * 128, :],
                                          Ao[:])
                else:
                    for rb in range(2):
                        rs = slice(rb * 128, (rb + 1) * 128)
                        psP3 = psum_mm.tile([128, 256], F32, tag="mm", name="psP3")
                        for kb in range(2):
                            nc.tensor.matmul(psP3[:], Xt[kb][:, rs], Wupd[2][kb][:],
                                             start=(kb == 0), stop=(kb == 1))
                        Po = outp.tile([128, 256], F32, tag="Pout", name="Pout")
                        nc.vector.tensor_copy(Po[:], psP3[:])
                        nc.sync.dma_start(P3_d[t * 256 + rb * 128:t * 256 + (rb + 1) * 128, :],
                                          Po[:])

    nc.compile()
    return nc


def _make_group_consts(n_group):
    G = 128 // n_group
    Gk = np.zeros((G, 128), np.float32)
    for g in range(G):
        Gk[g, g * n_group:(g + 1) * n_group] = 16.0 * MASK_C
    Gq = np.zeros((2, G, 256), np.float32)
    for b in range(2):
        for g in range(G):
            q0 = b * 128 + g * n_group
            Gq[b, g, q0:q0 + n_group] = 1.0
    return Gk, Gq


def _pack_bcol(bq, bk, bv, bo):
    """[128, 8] bias columns per layer: col p*2+fb."""
    out = np.zeros((L, 128, 8), np.float32)
    for l in range(L):
        for pi, b in enumerate((bq, bk, bv, bo)):
            for fb in range(2):
                out[l, :, pi * 2 + fb] = b[l, fb * 128:(fb + 1) * 128]
    return out


_PROGRAM_CACHE = {}


def _get_program(R):
    if R not in _PROGRAM_CACHE:
        _PROGRAM_CACHE[R] = build_program(R)
    return _PROGRAM_CACHE[R]


def kernel(co_feat_in, co_feat_con, co_feat_0, weight_in, weight_con,
           pe_v_W, pe_v_b, pe_e_W, pe_e_b,
           Wq_v, bq_v, Wk_v, bk_v, Wv_v, bv_v, Wo_v, bo_v,
           Wq_e, bq_e, Wk_e, bk_e, Wv_e, bv_e, Wo_e, bo_e,
           upd_W, upd_b, perm):
    f = np.asarray
    co_feat_in = f(co_feat_in, np.float32)
    co_feat_con = f(co_feat_con, np.float32)
    co_feat_0 = f(co_feat_0, np.float32)
    weight_in = f(weight_in, np.float32)
    weight_con = f(weight_con, np.float32)
    perm = np.asarray(perm)

    R = E // NCORES
    nc = _get_program(R)

    Gk_v, Gq_v = _make_group_consts(DV)
    Gk_e, Gq_e = _make_group_consts(DE)

    shared = {
        "peW_v": f(pe_v_W, np.float32), "peW_e": f(pe_e_W, np.float32),
        "peb_v": f(pe_v_b, np.float32), "peb_e": f(pe_e_b, np.float32),
        "W_v": np.stack([f(Wq_v, np.float32), f(Wk_v, np.float32),
                         f(Wv_v, np.float32), f(Wo_v, np.float32)], axis=1).copy(),
        "W_e": np.stack([f(Wq_e, np.float32), f(Wk_e, np.float32),
                         f(Wv_e, np.float32), f(Wo_e, np.float32)], axis=1).copy(),
        "Bcol_v": _pack_bcol(f(bq_v, np.float32), f(bk_v, np.float32),
                             f(bv_v, np.float32), f(bo_v, np.float32)),
        "Bcol_e": _pack_bcol(f(bq_e, np.float32), f(bk_e, np.float32),
                             f(bv_e, np.float32), f(bo_e, np.float32)),
        "bvbc_v": np.ascontiguousarray(
            np.broadcast_to(f(bv_v, np.float32)[:, None, :], (L, 128, D))),
        "bvbc_e": np.ascontiguousarray(
            np.broadcast_to(f(bv_e, np.float32)[:, None, :], (L, 128, D))),
        "W_upd": np.ascontiguousarray(
            f(upd_W, np.float32).reshape(4, D, D)),
        "updb_bc": np.ascontiguousarray(
            np.broadcast_to(f(upd_b, np.float32)[None, :], (128, D))),
        "Gk_v": Gk_v, "Gq_v": Gq_v, "Gk_e": Gk_e, "Gq_e": Gq_e,
        "ones1": np.ones((128, 128), np.float32),
    }

    in_maps = []
    for c in range(NCORES):
        rs = slice(c * R, (c + 1) * R)
        m = dict(shared)
        m["xvt"] = np.ascontiguousarray(co_feat_in[rs].T)
        m["wvt"] = np.ascontiguousarray(weight_in[rs].T)
        m["xet"] = np.ascontiguousarray(co_feat_con[rs].T)
        m["wet"] = np.ascontiguousarray(weight_con[rs].T)
        m["x0t"] = np.ascontiguousarray(co_feat_0[rs].T)
        in_maps.append(m)

    global _last_in_maps
    _last_in_maps = in_maps
    res = run_bass_kernel_spmd(nc, in_maps, core_ids=list(range(NCORES)))
    A = np.concatenate([res.results[c]["A"] for c in range(NCORES)], axis=0)
    P3 = np.concatenate([res.results[c]["P3"] for c in range(NCORES)], axis=0)

    inv_perm = np.argsort(perm)
    out_in = A + P3[inv_perm]
    return np.stack([out_in, out_in[perm]]).astype(np.float32)



# revision 19
# speedup vs baseline: 1.5582x; 1.5582x over previous
"""CoNHD GD-layer Trainium2 kernel (8-core SPMD, Bass/Tile), v2.

Math (see reference): two independent set-attention stacks over fixed-size
mailbox groups (v-side: 2048 nodes x 32, e-side: 4096 hyperedges x 16),
followed by a 4*D -> D update linear applied in two eid orders.

Device strategy (vs the v1 baseline):
  - Shard rows (E=65536) across 8 cores; core c owns rows [c*8192,(c+1)*8192).
  - All on-chip tensors bf16 (PSUM accumulation fp32): full PE rate at any
    moving size, 2x DVE rate, half the DMA bytes.  Host converts inputs.
  - Whole per-core inputs DMA'd once into resident SBUF tiles (10 big DMAs).
  - 512-row slabs; projections are N=512 matmuls.
  - Attention restricted to the block-diagonal 128x128 key/query tiles
    (groups never cross the 128 boundary); group masking enters as a single
    rank-G matmul over the whole 4-head score bank, removed by exp bias.
  - Softmax: denominators via ones-column matmuls into one [4,512] PSUM row
    tile, one DVE reciprocal per slab-layer, broadcast via a K=2 sel matmul,
    O = psO*RB + Q, Z = max(Wo-out,0) + O via one scalar_tensor_tensor.
  - Biases all folded for free (Act-copy per-partition bias, or ones-row
    matmuls into PSUM), so the kernel is correct for arbitrary bias values.
  - Update linear decomposed by column blocks as in v1:
      A  = co_in@W1 + co_v@W2 + co_0@W4 + b   ('in' eid order, on device)
      P3 = co_e@W3                            ('con' eid order, on device)
      out_in  = A + P3[inv_perm]              (host add + gather)
      out_con = out_in[perm]                  (host gather)

kernel(**inputs) takes the full unsharded inputs and returns [2, E, D] f32.
"""
import sys

if "/opt/trn_rl_repo" not in sys.path:
    sys.path.insert(0, "/opt/trn_rl_repo")

from contextlib import ExitStack

import ml_dtypes
import numpy as np

import concourse.mybir as mybir
import concourse.tile as tile
from concourse import bacc
from concourse.bass_utils import run_bass_kernel_spmd

F32 = mybir.dt.float32
BF16 = mybir.dt.bfloat16
AF = mybir.ActivationFunctionType
ALU = mybir.AluOpType
BF16_NP = ml_dtypes.bfloat16

N, DV, M, DE, E = 2048, 32, 4096, 16, 65536
D, WD, L, H = 256, 64, 2, 4
NCORES = 8
MASK_C = 30.0
SLAB = 512
R = E // NCORES
NSLAB = R // SLAB


def _sab_layer(nc, pools, X, C, l):
    """One SAB layer on a 512-row slab. X = [fb] of [128,512] bf16 (feat-major).
    C holds per-side consts. Returns Z tiles (same layout)."""
    sb, sbQKV, sbO, sbX, sbE, pmm, pS, pO, pRB = pools
    Wl = C["W"][l]          # Wl[p][kb] = [128,256] bf16 tiles, p in q,k,v,o
    Bcol = C["Bcol"][l]     # [128,4] f32: cols p*2+fb for p in (q,k)
    bvbc = C["bvbc"][l]     # [128,512] bf16 broadcast of bv (rb2-tiled)
    boS = C["boS"][l]       # [1,256] bf16
    Gk, Gq = C["Gk"], C["Gq"]
    ones64, onesrow, negc = C["ones64"], C["onesrow"], C["negc"]

    # Q (Act copy w/ bias), K (Pool copy w/ bias): feat-major [128,512]
    Qt, Kt = [], []
    for fb in range(2):
        psQ = pmm.tile([128, 512], F32, tag="mm", name="psQ")
        for kb in range(2):
            nc.tensor.matmul(psQ[:], Wl["q"][kb][:, fb * 128:(fb + 1) * 128],
                             X[kb][:], start=(kb == 0), stop=(kb == 1))
        qt = sbQKV.tile([128, 512], BF16, tag="Qt", name="Qt")
        nc.scalar.activation(qt[:], psQ[:], AF.Identity, bias=Bcol[:, fb:fb + 1])
        Qt.append(qt)
    for fb in range(2):
        psK = pmm.tile([128, 512], F32, tag="mm", name="psK")
        for kb in range(2):
            nc.tensor.matmul(psK[:], Wl["k"][kb][:, fb * 128:(fb + 1) * 128],
                             X[kb][:], start=(kb == 0), stop=(kb == 1))
        kt = sbQKV.tile([128, 512], BF16, tag="Kt", name="Kt")
        nc.gpsimd.tensor_scalar_add(kt[:], psK[:], Bcol[:, 2 + fb:3 + fb])
        Kt.append(kt)

    # V row-major: Vt[vv] packs rb=2vv (cols 0:256) and rb=2vv+1 (cols 256:512)
    Vt = []
    for vv in range(2):
        psV = pmm.tile([128, 512], F32, tag="mm", name="psV")
        for rb2 in range(2):
            rb = vv * 2 + rb2
            reg = psV[:, rb2 * 256:(rb2 + 1) * 256]
            for kb in range(2):
                nc.tensor.matmul(reg, X[kb][:, rb * 128:(rb + 1) * 128],
                                 Wl["v"][kb][:], start=(kb == 0), stop=(kb == 1),
                                 skip_group_check=True)
        vt = sbQKV.tile([128, 512], BF16, tag="Vt", name="Vt")
        nc.gpsimd.tensor_add(vt[:], psV[:], bvbc[:])
        Vt.append(vt)

    # scores (diag 128x128 blocks, 4 heads batched per PSUM bank) + exp
    eS = sbE.tile([128, 2048], BF16, tag="eS", name="eS")
    eSr = eS[:].rearrange("p (a b) -> p a b", a=4, b=512)
    for qb in range(4):
        qs = slice(qb * 128, (qb + 1) * 128)
        psS = pS.tile([128, 512], F32, tag="psS", name="psS")
        nc.tensor.matmul(psS[:], Gk[:], Gq[:], start=True, stop=False,
                         skip_group_check=True)
        for h in range(4):
            pr, off = h // 2, (h % 2) * 64
            nc.tensor.matmul(psS[:, h * 128:(h + 1) * 128],
                             Kt[pr][off:off + 64, qs], Qt[pr][off:off + 64, qs],
                             start=False, stop=True, skip_group_check=True)
        nc.scalar.activation(eS[:, qb * 512:(qb + 1) * 512], psS[:], AF.Exp,
                             bias=negc[:], scale=1.0 / 16.0)

    # denominators broadcast into O-layout [128f(2 heads), 512q] per fb
    psDEN = [pRB.tile([128, 512], F32, tag="psDEN", name="psDEN") for _ in range(2)]
    for h in range(4):
        fb, off = h // 2, (h % 2) * 64
        nc.tensor.matmul(psDEN[fb][off:off + 64, :],
                         ones64[:], eSr[:, :, h * 128:(h + 1) * 128],
                         start=True, stop=True, skip_group_check=True)

    # AV into O-layout psum [128f(2 heads), 512q] per fb
    psOf = [pO.tile([128, 512], F32, tag="psO", name="psO") for _ in range(2)]
    for qb in range(4):
        for h in range(4):
            fb, off = h // 2, (h % 2) * 64
            nc.tensor.matmul(
                psOf[fb][off:off + 64, qb * 128:(qb + 1) * 128],
                Vt[qb // 2][:, (qb % 2) * 256 + h * 64:(qb % 2) * 256 + h * 64 + 64],
                eS[:, qb * 512 + h * 128:qb * 512 + (h + 1) * 128],
                start=True, stop=True, skip_group_check=True)

    # O = psO * (1/den) + Q
    Ot = []
    for fb in range(2):
        rec = sb.tile([128, 512], BF16, tag="rec", name="rec")
        nc.vector.reciprocal(rec[:], psDEN[fb][:])
        U = sb.tile([128, 512], BF16, tag="Ut", name="Ut")
        nc.vector.tensor_mul(U[:], psOf[fb][:], rec[:])
        O = sbO.tile([128, 512], BF16, tag="Ot", name="Ot")
        nc.gpsimd.tensor_add(O[:], U[:], Qt[fb][:])
        Ot.append(O)

    # Z = O + relu(O @ Wo + bo)
    Zt = []
    for fb in range(2):
        psR = pS.tile([128, 512], F32, tag="psS", name="psR")
        for kb in range(2):
            nc.tensor.matmul(psR[:], Wl["o"][kb][:, fb * 128:(fb + 1) * 128],
                             Ot[kb][:], start=(kb == 0), stop=False,
                             skip_group_check=True)
        nc.tensor.matmul(psR[:], boS[0:1, fb * 128:(fb + 1) * 128], onesrow[:],
                         start=False, stop=True, skip_group_check=True)
        Z = sbX.tile([128, 512], BF16, tag="Xt", name="Zt")
        nc.vector.scalar_tensor_tensor(Z[:], psR[:], 0.0, Ot[fb][:],
                                       ALU.max, ALU.add)
        Zt.append(Z)
    return Zt


def _load_side_consts(nc, const, tag, dram, G):
    C = {}
    C["W"] = []
    for l in range(L):
        Wl = {}
        for pi, p in enumerate(["q", "k", "v", "o"]):
            Wl[p] = []
            for kb in range(2):
                t = const.tile([128, 256], BF16, tag=f"{tag}W{l}{p}{kb}",
                               name=f"{tag}W{l}{p}{kb}")
                nc.sync.dma_start(t[:], dram["W"][l, pi, kb * 128:(kb + 1) * 128, :])
                Wl[p].append(t)
        C["W"].append(Wl)
    C["Bcol"], C["bvbc"], C["boS"] = [], [], []
    for l in range(L):
        bc = const.tile([128, 4], F32, tag=f"{tag}Bcol{l}", name=f"{tag}Bcol{l}")
        nc.sync.dma_start(bc[:], dram["Bcol"][l])
        C["Bcol"].append(bc)
        bv = const.tile([128, 512], BF16, tag=f"{tag}bv{l}", name=f"{tag}bv{l}")
        nc.sync.dma_start(bv[:], dram["bvbc"][l])
        C["bvbc"].append(bv)
        bo = const.tile([1, 256], BF16, tag=f"{tag}bo{l}", name=f"{tag}bo{l}")
        nc.sync.dma_start(bo[:], dram["boS"][l])
        C["boS"].append(bo)
    gk = const.tile([G, 128], BF16, tag=f"{tag}Gk", name=f"{tag}Gk")
    nc.sync.dma_start(gk[:], dram["Gk"])
    C["Gk"] = gk
    gq = const.tile([G, 512], BF16, tag=f"{tag}Gq", name=f"{tag}Gq")
    nc.sync.dma_start(gq[:], dram["Gq"])
    C["Gq"] = gq
    return C


def build_program():
    nc = bacc.Bacc("TRN2", target_bir_lowering=False, debug=False)

    dram = {}

    def din(name, shape, dt=BF16):
        dram[name] = nc.dram_tensor(name, shape, dt, kind="ExternalInput").ap()
        return dram[name]

    xv_d = din("xv", [2, 128, R])
    wv_d = din("wv", [WD, R])
    xe_d = din("xe", [2, 128, R])
    we_d = din("we", [WD, R])
    x0_d = din("x0", [2, 128, R])
    peW_v_d = din("peW_v", [WD, D])
    peW_e_d = din("peW_e", [WD, D])
    peb_d = din("peb", [128, 4], F32)  # cols: side*2+fb
    W_v_d = din("W_v", [L, 4, D, D])
    W_e_d = din("W_e", [L, 4, D, D])
    Bcol_v_d = din("Bcol_v", [L, 128, 4], F32)
    Bcol_e_d = din("Bcol_e", [L, 128, 4], F32)
    bvbc_v_d = din("bvbc_v", [L, 128, 512])
    bvbc_e_d = din("bvbc_e", [L, 128, 512])
    boS_v_d = din("boS_v", [L, 1, D])
    boS_e_d = din("boS_e", [L, 1, D])
    Wupd_d = din("W_upd", [4, D, D])
    updb_d = din("updb_col", [128, 2], F32)
    Gk_v_d = din("Gk_v", [4, 128])
    Gq_v_d = din("Gq_v", [4, 512])
    Gk_e_d = din("Gk_e", [8, 128])
    Gq_e_d = din("Gq_e", [8, 512])
    ones64_d = din("ones64", [128, 64])
    onesrow_d = din("onesrow", [1, 512])

    A_d = nc.dram_tensor("A", [D, R], F32, kind="ExternalOutput").ap()
    P3_d = nc.dram_tensor("P3", [D, R], F32, kind="ExternalOutput").ap()

    with tile.TileContext(nc) as tc, ExitStack() as es, \
            nc.allow_low_precision(reason="bf16 pipeline, fp32 accum in PSUM"):
        const = es.enter_context(tc.tile_pool(name="const", bufs=1))
        resid = es.enter_context(tc.tile_pool(name="resid", bufs=1))
        sb = es.enter_context(tc.tile_pool(name="sb", bufs=3))
        sbQKV = es.enter_context(tc.tile_pool(name="sbQKV", bufs=3))
        sbO = es.enter_context(tc.tile_pool(name="sbO", bufs=4))
        sbX = es.enter_context(tc.tile_pool(name="sbX", bufs=4))
        sbE = es.enter_context(tc.tile_pool(name="sbE", bufs=4))
        outp = es.enter_context(tc.tile_pool(name="outp", bufs=2))
        pmm = es.enter_context(tc.tile_pool(name="pmm", bufs=2, space="PSUM"))
        pS = es.enter_context(tc.tile_pool(name="pS", bufs=2, space="PSUM"))
        pO = es.enter_context(tc.tile_pool(name="pO", bufs=2, space="PSUM"))
        pRB = es.enter_context(tc.tile_pool(name="pRB", bufs=2, space="PSUM"))
        pools = (sb, sbQKV, sbO, sbX, sbE, pmm, pS, pO, pRB)

        # resident inputs (order: v first so slab 0 can start early)
        xres, wres = {}, {}
        for s, x_d, w_d in (("v", xv_d, wv_d), ("e", xe_d, we_d)):
            xres[s] = []
            for fb in range(2):
                t = resid.tile([128, R], BF16, tag=f"x{s}{fb}", name=f"x{s}{fb}")
                nc.sync.dma_start(t[:], x_d[fb])
                xres[s].append(t)
            wt = resid.tile([WD, R], BF16, tag=f"w{s}", name=f"w{s}")
            nc.sync.dma_start(wt[:], w_d)
            wres[s] = wt
        x0res = []
        for fb in range(2):
            t = resid.tile([128, R], BF16, tag=f"x0{fb}", name=f"x0{fb}")
            nc.sync.dma_start(t[:], x0_d[fb])
            x0res.append(t)

        # consts
        negc = const.tile([128, 1], F32, tag="negc", name="negc")
        nc.vector.memset(negc[:], -MASK_C)
        ones64 = const.tile([128, 64], BF16, tag="ones64", name="ones64")
        nc.sync.dma_start(ones64[:], ones64_d)
        onesrow = const.tile([1, 512], BF16, tag="onesrow", name="onesrow")
        nc.sync.dma_start(onesrow[:], onesrow_d)
        peW, peb = {}, {}
        pebt = const.tile([128, 4], F32, tag="peb", name="peb")
        nc.sync.dma_start(pebt[:], peb_d)
        for si, (s, peW_d) in enumerate((("v", peW_v_d), ("e", peW_e_d))):
            t = const.tile([WD, D], BF16, tag=f"peW_{s}", name=f"peW_{s}")
            nc.sync.dma_start(t[:], peW_d)
            peW[s] = t
            peb[s] = pebt[:, si * 2:si * 2 + 2]

        side_consts = {}
        for s, W_d, Bc_d, bv_d, bo_d, Gk_d, Gq_d, G in (
                ("v", W_v_d, Bcol_v_d, bvbc_v_d, boS_v_d, Gk_v_d, Gq_v_d, 4),
                ("e", W_e_d, Bcol_e_d, bvbc_e_d, boS_e_d, Gk_e_d, Gq_e_d, 8)):
            side_consts[s] = _load_side_consts(
                nc, const, s,
                {"W": W_d, "Bcol": Bc_d, "bvbc": bv_d, "boS": bo_d,
                 "Gk": Gk_d, "Gq": Gq_d}, G)
            side_consts[s]["ones64"] = ones64
            side_consts[s]["onesrow"] = onesrow
            side_consts[s]["negc"] = negc

        Wupd = []
        for j in range(4):
            Wupd.append([])
            for kb in range(2):
                t = const.tile([128, 256], BF16, tag=f"Wu{j}{kb}", name=f"Wu{j}{kb}")
                nc.sync.dma_start(t[:], Wupd_d[j, kb * 128:(kb + 1) * 128, :])
                Wupd[j].append(t)
        updbcol = const.tile([128, 2], F32, tag="updb", name="updb")
        nc.sync.dma_start(updbcol[:], updb_d)

        for side in ("v", "e"):
            C = side_consts[side]
            for t in range(NSLAB):
                cs = slice(t * SLAB, (t + 1) * SLAB)
                # mailbox: X = x + peW^T w + peb
                X = []
                for fb in range(2):
                    psP = pmm.tile([128, 512], F32, tag="mm", name="psP")
                    nc.tensor.matmul(psP[:], peW[side][:, fb * 128:(fb + 1) * 128],
                                     wres[side][:, cs], start=True, stop=True)
                    xt = sbX.tile([128, 512], BF16, tag="Xt", name="Xt")
                    nc.vector.scalar_tensor_tensor(
                        xt[:], psP[:], peb[side][:, fb:fb + 1],
                        xres[side][fb][:, cs], ALU.add, ALU.add)
                    X.append(xt)

                for l in range(L):
                    X = _sab_layer(nc, pools, X, C, l)

                if side == "v":
                    # A^T[fb] = sum_j Wupd_j[:,fb]^T @ src_j + updb (feat-major)
                    Ao = outp.tile([128, 1024], F32, tag="Out", name="Aout")
                    for fb in range(2):
                        psA = pS.tile([128, 512], F32, tag="psS", name="psA")
                        first = True
                        for srcs, j in ((None, 0), (X, 1), (None, 3)):
                            for kb in range(2):
                                if srcs is None:
                                    base = xres["v"] if j == 0 else x0res
                                    rhs = base[kb][:, cs]
                                else:
                                    rhs = srcs[kb][:]
                                nc.tensor.matmul(
                                    psA[:], Wupd[j][kb][:, fb * 128:(fb + 1) * 128],
                                    rhs, start=first, stop=(j == 3 and kb == 1),
                                    skip_group_check=True)
                                first = False
                        if fb == 0:
                            nc.vector.tensor_scalar_add(
                                Ao[:, 0:512], psA[:], updbcol[:, 0:1])
                        else:
                            nc.scalar.activation(
                                Ao[:, 512:1024], psA[:], AF.Identity,
                                bias=updbcol[:, 1:2])
                    nc.sync.dma_start(
                        A_d[:, cs].rearrange("(a p) r -> p a r", a=2, p=128),
                        Ao[:].rearrange("p (a r) -> p a r", a=2, r=512))
                else:
                    Po = outp.tile([128, 1024], F32, tag="Out", name="Pout")
                    for fb in range(2):
                        psP3 = pS.tile([128, 512], F32, tag="psS", name="psP3")
                        for kb in range(2):
                            nc.tensor.matmul(
                                psP3[:], Wupd[2][kb][:, fb * 128:(fb + 1) * 128],
                                X[kb][:], start=(kb == 0), stop=(kb == 1),
                                skip_group_check=True)
                        if fb == 0:
                            nc.vector.tensor_copy(Po[:, 0:512], psP3[:])
                        else:
                            nc.scalar.copy(Po[:, 512:1024], psP3[:])
                    nc.sync.dma_start(
                        P3_d[:, cs].rearrange("(a p) r -> p a r", a=2, p=128),
                        Po[:].rearrange("p (a r) -> p a r", a=2, r=512))

    nc.compile()
    return nc


def _make_group_consts(n_group):
    G = 128 // n_group
    Gk = np.zeros((G, 128), BF16_NP)
    for g in range(G):
        Gk[g, g * n_group:(g + 1) * n_group] = 16.0 * MASK_C
    Gq = np.zeros((G, 512), BF16_NP)
    for h in range(4):
        for g in range(G):
            q0 = h * 128 + g * n_group
            Gq[g, q0:q0 + n_group] = 1.0
    return Gk, Gq


_PROGRAM_CACHE = {}


def _get_program(_r=R):
    if _r not in _PROGRAM_CACHE:
        _PROGRAM_CACHE[_r] = build_program()
    return _PROGRAM_CACHE[_r]


def kernel(co_feat_in, co_feat_con, co_feat_0, weight_in, weight_con,
           pe_v_W, pe_v_b, pe_e_W, pe_e_b,
           Wq_v, bq_v, Wk_v, bk_v, Wv_v, bv_v, Wo_v, bo_v,
           Wq_e, bq_e, Wk_e, bk_e, Wv_e, bv_e, Wo_e, bo_e,
           upd_W, upd_b, perm):
    f32 = lambda x: np.asarray(x, np.float32)
    bf = lambda x: np.asarray(x).astype(BF16_NP)
    perm = np.asarray(perm)

    nc = _get_program()

    Gk_v, Gq_v = _make_group_consts(DV)
    Gk_e, Gq_e = _make_group_consts(DE)

    def bcol(bq, bk):
        out = np.zeros((L, 128, 4), np.float32)
        for l in range(L):
            for pi, b in enumerate((f32(bq), f32(bk))):
                for fb in range(2):
                    out[l, :, pi * 2 + fb] = b[l, fb * 128:(fb + 1) * 128]
        return out

    peb = np.zeros((128, 4), np.float32)
    for si, b in enumerate((f32(pe_v_b), f32(pe_e_b))):
        for fb in range(2):
            peb[:, si * 2 + fb] = b[fb * 128:(fb + 1) * 128]

    shared = {
        "peW_v": bf(pe_v_W), "peW_e": bf(pe_e_W), "peb": peb,
        "W_v": np.stack([bf(Wq_v), bf(Wk_v), bf(Wv_v), bf(Wo_v)], axis=1).copy(),
        "W_e": np.stack([bf(Wq_e), bf(Wk_e), bf(Wv_e), bf(Wo_e)], axis=1).copy(),
        "Bcol_v": bcol(bq_v, bk_v), "Bcol_e": bcol(bq_e, bk_e),
        "bvbc_v": np.ascontiguousarray(np.broadcast_to(
            np.tile(bf(bv_v), (1, 2))[:, None, :], (L, 128, 512))),
        "bvbc_e": np.ascontiguousarray(np.broadcast_to(
            np.tile(bf(bv_e), (1, 2))[:, None, :], (L, 128, 512))),
        "boS_v": bf(bo_v).reshape(L, 1, D).copy(),
        "boS_e": bf(bo_e).reshape(L, 1, D).copy(),
        "W_upd": bf(upd_W).reshape(4, D, D).copy(),
        "updb_col": f32(upd_b).reshape(2, 128).T.copy(),
        "Gk_v": Gk_v, "Gq_v": Gq_v, "Gk_e": Gk_e, "Gq_e": Gq_e,
        "ones64": np.ones((128, 64), BF16_NP),
        "onesrow": np.ones((1, 512), BF16_NP),
    }

    def xsplit(a, rs):
        t = np.ascontiguousarray(np.asarray(a)[rs].T.astype(BF16_NP))
        return t.reshape(2, 128, R)

    in_maps = []
    for c in range(NCORES):
        rs = slice(c * R, (c + 1) * R)
        m = dict(shared)
        m["xv"] = xsplit(co_feat_in, rs)
        m["wv"] = np.ascontiguousarray(np.asarray(weight_in)[rs].T.astype(BF16_NP))
        m["xe"] = xsplit(co_feat_con, rs)
        m["we"] = np.ascontiguousarray(np.asarray(weight_con)[rs].T.astype(BF16_NP))
        m["x0"] = xsplit(co_feat_0, rs)
        in_maps.append(m)

    res = run_bass_kernel_spmd(nc, in_maps, core_ids=list(range(NCORES)))
    A = np.concatenate([res.results[c]["A"].T for c in range(NCORES)], axis=0)
    P3 = np.concatenate([res.results[c]["P3"].T for c in range(NCORES)], axis=0)

    inv_perm = np.argsort(perm)
    out_in = A + P3[inv_perm]
    return np.stack([out_in, out_in[perm]]).astype(np.float32)


# revision 20
# speedup vs baseline: 1.7452x; 1.1200x over previous
"""CoNHD GD-layer Trainium2 kernel (8-core SPMD, Bass/Tile), v2.

Math (see reference): two independent set-attention stacks over fixed-size
mailbox groups (v-side: 2048 nodes x 32, e-side: 4096 hyperedges x 16),
followed by a 4*D -> D update linear applied in two eid orders.

Device strategy (vs the v1 baseline):
  - Shard rows (E=65536) across 8 cores; core c owns rows [c*8192,(c+1)*8192).
  - All on-chip tensors bf16 (PSUM accumulation fp32): full PE rate at any
    moving size, 2x DVE rate, half the DMA bytes.  Host converts inputs.
  - Whole per-core inputs DMA'd once into resident SBUF tiles (10 big DMAs).
  - 512-row slabs; projections are N=512 matmuls.
  - Attention restricted to the block-diagonal 128x128 key/query tiles
    (groups never cross the 128 boundary); group masking enters as a single
    rank-G matmul over the whole 4-head score bank, removed by exp bias.
  - Softmax: denominators via ones-column matmuls into one [4,512] PSUM row
    tile, one DVE reciprocal per slab-layer, broadcast via a K=2 sel matmul,
    O = psO*RB + Q, Z = max(Wo-out,0) + O via one scalar_tensor_tensor.
  - Biases all folded for free (Act-copy per-partition bias, or ones-row
    matmuls into PSUM), so the kernel is correct for arbitrary bias values.
  - Update linear decomposed by column blocks as in v1:
      A  = co_in@W1 + co_v@W2 + co_0@W4 + b   ('in' eid order, on device)
      P3 = co_e@W3                            ('con' eid order, on device)
      out_in  = A + P3[inv_perm]              (host add + gather)
      out_con = out_in[perm]                  (host gather)

kernel(**inputs) takes the full unsharded inputs and returns [2, E, D] f32.
"""
import sys

if "/opt/trn_rl_repo" not in sys.path:
    sys.path.insert(0, "/opt/trn_rl_repo")

from contextlib import ExitStack

import ml_dtypes
import numpy as np

import concourse.mybir as mybir
import concourse.tile as tile
from concourse import bacc
from concourse.bass_utils import run_bass_kernel_spmd

F32 = mybir.dt.float32
BF16 = mybir.dt.bfloat16
AF = mybir.ActivationFunctionType
ALU = mybir.AluOpType
BF16_NP = ml_dtypes.bfloat16

N, DV, M, DE, E = 2048, 32, 4096, 16, 65536
D, WD, L, H = 256, 64, 2, 4
NCORES = 8
MASK_C = 30.0
SLAB = 512
R = E // NCORES
NSLAB = R // SLAB


def _sab_layer(nc, pools, X, C, l):
    """One SAB layer on a 512-row slab. X = [fb] of [128,512] bf16 (feat-major).
    C holds per-side consts. Returns Z tiles (same layout)."""
    sb, sbQKV, sbO, sbX, sbE, pmm, pS, pO, pRB = pools
    Wl = C["W"][l]          # Wl[p][kb] = [128,256] bf16 tiles, p in q,k,v,o
    Bcol = C["Bcol"][l]     # [128,4] f32: cols p*2+fb for p in (q,k)
    bvbc = C["bvbc"][l]     # [128,512] bf16 broadcast of bv (rb2-tiled)
    boS = C["boS"][l]       # [1,256] bf16
    Gk, Gq = C["Gk"], C["Gq"]
    ones64, onesrow, negc = C["ones64"], C["onesrow"], C["negc"]

    # Q (Act copy w/ bias), K (Pool copy w/ bias): feat-major [128,512]
    Qt, Kt = [], []
    for fb in range(2):
        psQ = pmm.tile([128, 512], F32, tag="mm", name="psQ")
        for kb in range(2):
            nc.tensor.matmul(psQ[:], Wl["q"][kb][:, fb * 128:(fb + 1) * 128],
                             X[kb][:], start=(kb == 0), stop=(kb == 1))
        qt = sbQKV.tile([128, 512], BF16, tag="Qt", name="Qt")
        nc.scalar.activation(qt[:], psQ[:], AF.Identity, bias=Bcol[:, fb:fb + 1])
        Qt.append(qt)
    for fb in range(2):
        psK = pmm.tile([128, 512], F32, tag="mm", name="psK")
        for kb in range(2):
            nc.tensor.matmul(psK[:], Wl["k"][kb][:, fb * 128:(fb + 1) * 128],
                             X[kb][:], start=(kb == 0), stop=(kb == 1))
        kt = sbQKV.tile([128, 512], BF16, tag="Kt", name="Kt")
        nc.gpsimd.tensor_scalar_add(kt[:], psK[:], Bcol[:, 2 + fb:3 + fb])
        Kt.append(kt)

    # V row-major: Vt[vv] packs rb=2vv (cols 0:256) and rb=2vv+1 (cols 256:512)
    Vt = []
    for vv in range(2):
        psV = pmm.tile([128, 512], F32, tag="mm", name="psV")
        for rb2 in range(2):
            rb = vv * 2 + rb2
            reg = psV[:, rb2 * 256:(rb2 + 1) * 256]
            for kb in range(2):
                nc.tensor.matmul(reg, X[kb][:, rb * 128:(rb + 1) * 128],
                                 Wl["v"][kb][:], start=(kb == 0), stop=(kb == 1),
                                 skip_group_check=True)
        vt = sbQKV.tile([128, 512], BF16, tag="Vt", name="Vt")
        nc.gpsimd.tensor_add(vt[:], psV[:], bvbc[:])
        Vt.append(vt)

    # scores (diag 128x128 blocks, 4 heads batched per PSUM bank) + exp
    eS = sbE.tile([128, 2048], BF16, tag="eS", name="eS")
    eSr = eS[:].rearrange("p (a b) -> p a b", a=4, b=512)
    for qb in range(4):
        qs = slice(qb * 128, (qb + 1) * 128)
        psS = pS.tile([128, 512], F32, tag="psS", name="psS")
        nc.tensor.matmul(psS[:], Gk[:], Gq[:], start=True, stop=False,
                         skip_group_check=True)
        for h in range(4):
            pr, off = h // 2, (h % 2) * 64
            nc.tensor.matmul(psS[:, h * 128:(h + 1) * 128],
                             Kt[pr][off:off + 64, qs], Qt[pr][off:off + 64, qs],
                             start=False, stop=True, skip_group_check=True)
        nc.scalar.activation(eS[:, qb * 512:(qb + 1) * 512], psS[:], AF.Exp,
                             bias=negc[:], scale=1.0 / 16.0)

    # denominators broadcast into O-layout [128f(2 heads), 512q] per fb
    psDEN = [pRB.tile([128, 512], F32, tag="psDEN", name="psDEN") for _ in range(2)]
    for h in range(4):
        fb, off = h // 2, (h % 2) * 64
        nc.tensor.matmul(psDEN[fb][off:off + 64, :],
                         ones64[:], eSr[:, :, h * 128:(h + 1) * 128],
                         start=True, stop=True, skip_group_check=True)

    # AV into O-layout psum [128f(2 heads), 512q] per fb
    psOf = [pO.tile([128, 512], F32, tag="psO", name="psO") for _ in range(2)]
    for qb in range(4):
        for h in range(4):
            fb, off = h // 2, (h % 2) * 64
            nc.tensor.matmul(
                psOf[fb][off:off + 64, qb * 128:(qb + 1) * 128],
                Vt[qb // 2][:, (qb % 2) * 256 + h * 64:(qb % 2) * 256 + h * 64 + 64],
                eS[:, qb * 512 + h * 128:qb * 512 + (h + 1) * 128],
                start=True, stop=True, skip_group_check=True)

    # O = psO / den + Q
    Ot = []
    for fb in range(2):
        U = sb.tile([128, 512], BF16, tag="Ut", name="Ut")
        nc.vector.tensor_tensor(U[:], psOf[fb][:], psDEN[fb][:], ALU.divide)
        O = sbO.tile([128, 512], BF16, tag="Ot", name="Ot")
        nc.vector.tensor_add(O[:], U[:], Qt[fb][:])
        Ot.append(O)

    # Z = O + relu(O @ Wo + bo)
    Zt = []
    for fb in range(2):
        psR = pS.tile([128, 512], F32, tag="psS", name="psR")
        for kb in range(2):
            nc.tensor.matmul(psR[:], Wl["o"][kb][:, fb * 128:(fb + 1) * 128],
                             Ot[kb][:], start=(kb == 0), stop=False,
                             skip_group_check=True)
        nc.tensor.matmul(psR[:], boS[0:1, fb * 128:(fb + 1) * 128], onesrow[:],
                         start=False, stop=True, skip_group_check=True)
        Z = sbX.tile([128, 512], BF16, tag="Xt", name="Zt")
        nc.vector.scalar_tensor_tensor(Z[:], psR[:], 0.0, Ot[fb][:],
                                       ALU.max, ALU.add)
        Zt.append(Z)
    return Zt


def _load_side_consts(nc, const, tag, dram, G):
    C = {}
    C["W"] = []
    for l in range(L):
        Wl = {}
        for pi, p in enumerate(["q", "k", "v", "o"]):
            Wl[p] = []
            for kb in range(2):
                t = const.tile([128, 256], BF16, tag=f"{tag}W{l}{p}{kb}",
                               name=f"{tag}W{l}{p}{kb}")
                nc.sync.dma_start(t[:], dram["W"][l, pi, kb * 128:(kb + 1) * 128, :])
                Wl[p].append(t)
        C["W"].append(Wl)
    C["Bcol"], C["bvbc"], C["boS"] = [], [], []
    for l in range(L):
        bc = const.tile([128, 4], F32, tag=f"{tag}Bcol{l}", name=f"{tag}Bcol{l}")
        nc.sync.dma_start(bc[:], dram["Bcol"][l])
        C["Bcol"].append(bc)
        bv = const.tile([128, 512], BF16, tag=f"{tag}bv{l}", name=f"{tag}bv{l}")
        nc.sync.dma_start(bv[:], dram["bvbc"][l])
        C["bvbc"].append(bv)
        bo = const.tile([1, 256], BF16, tag=f"{tag}bo{l}", name=f"{tag}bo{l}")
        nc.sync.dma_start(bo[:], dram["boS"][l])
        C["boS"].append(bo)
    gk = const.tile([G, 128], BF16, tag=f"{tag}Gk", name=f"{tag}Gk")
    nc.sync.dma_start(gk[:], dram["Gk"])
    C["Gk"] = gk
    gq = const.tile([G, 512], BF16, tag=f"{tag}Gq", name=f"{tag}Gq")
    nc.sync.dma_start(gq[:], dram["Gq"])
    C["Gq"] = gq
    return C


def build_program():
    nc = bacc.Bacc("TRN2", target_bir_lowering=False, debug=False)

    dram = {}

    def din(name, shape, dt=BF16):
        dram[name] = nc.dram_tensor(name, shape, dt, kind="ExternalInput").ap()
        return dram[name]

    xv_d = din("xv", [2, 128, R])
    wv_d = din("wv", [WD, R])
    xe_d = din("xe", [2, 128, R])
    we_d = din("we", [WD, R])
    x0_d = din("x0", [2, 128, R])
    peW_v_d = din("peW_v", [WD, D])
    peW_e_d = din("peW_e", [WD, D])
    peb_d = din("peb", [128, 4], F32)  # cols: side*2+fb
    W_v_d = din("W_v", [L, 4, D, D])
    W_e_d = din("W_e", [L, 4, D, D])
    Bcol_v_d = din("Bcol_v", [L, 128, 4], F32)
    Bcol_e_d = din("Bcol_e", [L, 128, 4], F32)
    bvbc_v_d = din("bvbc_v", [L, 128, 512])
    bvbc_e_d = din("bvbc_e", [L, 128, 512])
    boS_v_d = din("boS_v", [L, 1, D])
    boS_e_d = din("boS_e", [L, 1, D])
    Wupd_d = din("W_upd", [4, D, D])
    updb_d = din("updb_col", [128, 2], F32)
    Gk_v_d = din("Gk_v", [4, 128])
    Gq_v_d = din("Gq_v", [4, 512])
    Gk_e_d = din("Gk_e", [8, 128])
    Gq_e_d = din("Gq_e", [8, 512])
    ones64_d = din("ones64", [128, 64])
    onesrow_d = din("onesrow", [1, 512])

    A_d = nc.dram_tensor("A", [D, R], F32, kind="ExternalOutput").ap()
    P3_d = nc.dram_tensor("P3", [D, R], F32, kind="ExternalOutput").ap()

    with tile.TileContext(nc) as tc, ExitStack() as es, \
            nc.allow_low_precision(reason="bf16 pipeline, fp32 accum in PSUM"):
        const = es.enter_context(tc.tile_pool(name="const", bufs=1))
        resid = es.enter_context(tc.tile_pool(name="resid", bufs=1))
        sb = es.enter_context(tc.tile_pool(name="sb", bufs=3))
        sbQKV = es.enter_context(tc.tile_pool(name="sbQKV", bufs=3))
        sbO = es.enter_context(tc.tile_pool(name="sbO", bufs=4))
        sbX = es.enter_context(tc.tile_pool(name="sbX", bufs=4))
        sbE = es.enter_context(tc.tile_pool(name="sbE", bufs=4))
        outp = es.enter_context(tc.tile_pool(name="outp", bufs=2))
        pmm = es.enter_context(tc.tile_pool(name="pmm", bufs=2, space="PSUM"))
        pS = es.enter_context(tc.tile_pool(name="pS", bufs=2, space="PSUM"))
        pO = es.enter_context(tc.tile_pool(name="pO", bufs=2, space="PSUM"))
        pRB = es.enter_context(tc.tile_pool(name="pRB", bufs=2, space="PSUM"))
        pools = (sb, sbQKV, sbO, sbX, sbE, pmm, pS, pO, pRB)

        # resident inputs (order: v first so slab 0 can start early)
        xres, wres = {}, {}
        for s, x_d, w_d in (("v", xv_d, wv_d), ("e", xe_d, we_d)):
            xres[s] = []
            for fb in range(2):
                t = resid.tile([128, R], BF16, tag=f"x{s}{fb}", name=f"x{s}{fb}")
                nc.sync.dma_start(t[:], x_d[fb])
                xres[s].append(t)
            wt = resid.tile([WD, R], BF16, tag=f"w{s}", name=f"w{s}")
            nc.sync.dma_start(wt[:], w_d)
            wres[s] = wt
        x0res = []
        for fb in range(2):
            t = resid.tile([128, R], BF16, tag=f"x0{fb}", name=f"x0{fb}")
            nc.sync.dma_start(t[:], x0_d[fb])
            x0res.append(t)

        # consts
        negc = const.tile([128, 1], F32, tag="negc", name="negc")
        nc.vector.memset(negc[:], -MASK_C)
        ones64 = const.tile([128, 64], BF16, tag="ones64", name="ones64")
        nc.sync.dma_start(ones64[:], ones64_d)
        onesrow = const.tile([1, 512], BF16, tag="onesrow", name="onesrow")
        nc.sync.dma_start(onesrow[:], onesrow_d)
        peW, peb = {}, {}
        pebt = const.tile([128, 4], F32, tag="peb", name="peb")
        nc.sync.dma_start(pebt[:], peb_d)
        for si, (s, peW_d) in enumerate((("v", peW_v_d), ("e", peW_e_d))):
            t = const.tile([WD, D], BF16, tag=f"peW_{s}", name=f"peW_{s}")
            nc.sync.dma_start(t[:], peW_d)
            peW[s] = t
            peb[s] = pebt[:, si * 2:si * 2 + 2]

        side_consts = {}
        for s, W_d, Bc_d, bv_d, bo_d, Gk_d, Gq_d, G in (
                ("v", W_v_d, Bcol_v_d, bvbc_v_d, boS_v_d, Gk_v_d, Gq_v_d, 4),
                ("e", W_e_d, Bcol_e_d, bvbc_e_d, boS_e_d, Gk_e_d, Gq_e_d, 8)):
            side_consts[s] = _load_side_consts(
                nc, const, s,
                {"W": W_d, "Bcol": Bc_d, "bvbc": bv_d, "boS": bo_d,
                 "Gk": Gk_d, "Gq": Gq_d}, G)
            side_consts[s]["ones64"] = ones64
            side_consts[s]["onesrow"] = onesrow
            side_consts[s]["negc"] = negc

        Wupd = []
        for j in range(4):
            Wupd.append([])
            for kb in range(2):
                t = const.tile([128, 256], BF16, tag=f"Wu{j}{kb}", name=f"Wu{j}{kb}")
                nc.sync.dma_start(t[:], Wupd_d[j, kb * 128:(kb + 1) * 128, :])
                Wupd[j].append(t)
        updbcol = const.tile([128, 2], F32, tag="updb", name="updb")
        nc.sync.dma_start(updbcol[:], updb_d)

        for t in range(NSLAB):
            for side in ("v", "e"):
                C = side_consts[side]
                cs = slice(t * SLAB, (t + 1) * SLAB)
                # mailbox: X = x + peW^T w + peb
                X = []
                for fb in range(2):
                    psP = pmm.tile([128, 512], F32, tag="mm", name="psP")
                    nc.tensor.matmul(psP[:], peW[side][:, fb * 128:(fb + 1) * 128],
                                     wres[side][:, cs], start=True, stop=True)
                    xt = sbX.tile([128, 512], BF16, tag="Xt", name="Xt")
                    nc.vector.scalar_tensor_tensor(
                        xt[:], psP[:], peb[side][:, fb:fb + 1],
                        xres[side][fb][:, cs], ALU.add, ALU.add)
                    X.append(xt)

                for l in range(L):
                    X = _sab_layer(nc, pools, X, C, l)

                if side == "v":
                    # A^T[fb] = sum_j Wupd_j[:,fb]^T @ src_j + updb (feat-major)
                    Ao = outp.tile([128, 1024], F32, tag="Out", name="Aout")
                    for fb in range(2):
                        psA = pS.tile([128, 512], F32, tag="psS", name="psA")
                        first = True
                        for srcs, j in ((None, 0), (X, 1), (None, 3)):
                            for kb in range(2):
                                if srcs is None:
                                    base = xres["v"] if j == 0 else x0res
                                    rhs = base[kb][:, cs]
                                else:
                                    rhs = srcs[kb][:]
                                nc.tensor.matmul(
                                    psA[:], Wupd[j][kb][:, fb * 128:(fb + 1) * 128],
                                    rhs, start=first, stop=(j == 3 and kb == 1),
                                    skip_group_check=True)
                                first = False
                        if fb == 0:
                            nc.vector.tensor_scalar_add(
                                Ao[:, 0:512], psA[:], updbcol[:, 0:1])
                        else:
                            nc.scalar.activation(
                                Ao[:, 512:1024], psA[:], AF.Identity,
                                bias=updbcol[:, 1:2])
                    nc.sync.dma_start(
                        A_d[:, cs].rearrange("(a p) r -> p a r", a=2, p=128),
                        Ao[:].rearrange("p (a r) -> p a r", a=2, r=512))
                else:
                    Po = outp.tile([128, 1024], F32, tag="Out", name="Pout")
                    for fb in range(2):
                        psP3 = pS.tile([128, 512], F32, tag="psS", name="psP3")
                        for kb in range(2):
                            nc.tensor.matmul(
                                psP3[:], Wupd[2][kb][:, fb * 128:(fb + 1) * 128],
                                X[kb][:], start=(kb == 0), stop=(kb == 1),
                                skip_group_check=True)
                        if fb == 0:
                            nc.vector.tensor_copy(Po[:, 0:512], psP3[:])
                        else:
                            nc.scalar.copy(Po[:, 512:1024], psP3[:])
                    nc.sync.dma_start(
                        P3_d[:, cs].rearrange("(a p) r -> p a r", a=2, p=128),
                        Po[:].rearrange("p (a r) -> p a r", a=2, r=512))

    nc.compile()
    return nc


def _make_group_consts(n_group):
    G = 128 // n_group
    Gk = np.zeros((G, 128), BF16_NP)
    for g in range(G):
        Gk[g, g * n_group:(g + 1) * n_group] = 16.0 * MASK_C
    Gq = np.zeros((G, 512), BF16_NP)
    for h in range(4):
        for g in range(G):
            q0 = h * 128 + g * n_group
            Gq[g, q0:q0 + n_group] = 1.0
    return Gk, Gq


_PROGRAM_CACHE = {}


def _get_program(_r=R):
    if _r not in _PROGRAM_CACHE:
        _PROGRAM_CACHE[_r] = build_program()
    return _PROGRAM_CACHE[_r]


def kernel(co_feat_in, co_feat_con, co_feat_0, weight_in, weight_con,
           pe_v_W, pe_v_b, pe_e_W, pe_e_b,
           Wq_v, bq_v, Wk_v, bk_v, Wv_v, bv_v, Wo_v, bo_v,
           Wq_e, bq_e, Wk_e, bk_e, Wv_e, bv_e, Wo_e, bo_e,
           upd_W, upd_b, perm):
    f32 = lambda x: np.asarray(x, np.float32)
    bf = lambda x: np.asarray(x).astype(BF16_NP)
    perm = np.asarray(perm)

    nc = _get_program()

    Gk_v, Gq_v = _make_group_consts(DV)
    Gk_e, Gq_e = _make_group_consts(DE)

    def bcol(bq, bk):
        out = np.zeros((L, 128, 4), np.float32)
        for l in range(L):
            for pi, b in enumerate((f32(bq), f32(bk))):
                for fb in range(2):
                    out[l, :, pi * 2 + fb] = b[l, fb * 128:(fb + 1) * 128]
        return out

    peb = np.zeros((128, 4), np.float32)
    for si, b in enumerate((f32(pe_v_b), f32(pe_e_b))):
        for fb in range(2):
            peb[:, si * 2 + fb] = b[fb * 128:(fb + 1) * 128]

    shared = {
        "peW_v": bf(pe_v_W), "peW_e": bf(pe_e_W), "peb": peb,
        "W_v": np.stack([bf(Wq_v), bf(Wk_v), bf(Wv_v), bf(Wo_v)], axis=1).copy(),
        "W_e": np.stack([bf(Wq_e), bf(Wk_e), bf(Wv_e), bf(Wo_e)], axis=1).copy(),
        "Bcol_v": bcol(bq_v, bk_v), "Bcol_e": bcol(bq_e, bk_e),
        "bvbc_v": np.ascontiguousarray(np.broadcast_to(
            np.tile(bf(bv_v), (1, 2))[:, None, :], (L, 128, 512))),
        "bvbc_e": np.ascontiguousarray(np.broadcast_to(
            np.tile(bf(bv_e), (1, 2))[:, None, :], (L, 128, 512))),
        "boS_v": bf(bo_v).reshape(L, 1, D).copy(),
        "boS_e": bf(bo_e).reshape(L, 1, D).copy(),
        "W_upd": bf(upd_W).reshape(4, D, D).copy(),
        "updb_col": f32(upd_b).reshape(2, 128).T.copy(),
        "Gk_v": Gk_v, "Gq_v": Gq_v, "Gk_e": Gk_e, "Gq_e": Gq_e,
        "ones64": np.ones((128, 64), BF16_NP),
        "onesrow": np.ones((1, 512), BF16_NP),
    }

    def xsplit(a, rs):
        t = np.ascontiguousarray(np.asarray(a)[rs].T.astype(BF16_NP))
        return t.reshape(2, 128, R)

    in_maps = []
    for c in range(NCORES):
        rs = slice(c * R, (c + 1) * R)
        m = dict(shared)
        m["xv"] = xsplit(co_feat_in, rs)
        m["wv"] = np.ascontiguousarray(np.asarray(weight_in)[rs].T.astype(BF16_NP))
        m["xe"] = xsplit(co_feat_con, rs)
        m["we"] = np.ascontiguousarray(np.asarray(weight_con)[rs].T.astype(BF16_NP))
        m["x0"] = xsplit(co_feat_0, rs)
        in_maps.append(m)

    res = run_bass_kernel_spmd(nc, in_maps, core_ids=list(range(NCORES)))
    A = np.concatenate([res.results[c]["A"].T for c in range(NCORES)], axis=0)
    P3 = np.concatenate([res.results[c]["P3"].T for c in range(NCORES)], axis=0)

    inv_perm = np.argsort(perm)
    out_in = A + P3[inv_perm]
    return np.stack([out_in, out_in[perm]]).astype(np.float32)


# revision 23
# speedup vs baseline: 1.8076x; 1.0358x over previous
"""CoNHD GD-layer Trainium2 kernel (8-core SPMD, Bass/Tile), v2.

Math (see reference): two independent set-attention stacks over fixed-size
mailbox groups (v-side: 2048 nodes x 32, e-side: 4096 hyperedges x 16),
followed by a 4*D -> D update linear applied in two eid orders.

Device strategy (vs the v1 baseline):
  - Shard rows (E=65536) across 8 cores; core c owns rows [c*8192,(c+1)*8192).
  - All on-chip tensors bf16 (PSUM accumulation fp32): full PE rate at any
    moving size, 2x DVE rate, half the DMA bytes.  Host converts inputs.
  - Whole per-core inputs DMA'd once into resident SBUF tiles (10 big DMAs).
  - 512-row slabs; projections are N=512 matmuls.
  - Attention restricted to the block-diagonal 128x128 key/query tiles
    (groups never cross the 128 boundary); group masking enters as a single
    rank-G matmul over the whole 4-head score bank, removed by exp bias.
  - Softmax: denominators via ones-column matmuls into one [4,512] PSUM row
    tile, one DVE reciprocal per slab-layer, broadcast via a K=2 sel matmul,
    O = psO*RB + Q, Z = max(Wo-out,0) + O via one scalar_tensor_tensor.
  - Biases all folded for free (Act-copy per-partition bias, or ones-row
    matmuls into PSUM), so the kernel is correct for arbitrary bias values.
  - Update linear decomposed by column blocks as in v1:
      A  = co_in@W1 + co_v@W2 + co_0@W4 + b   ('in' eid order, on device)
      P3 = co_e@W3                            ('con' eid order, on device)
      out_in  = A + P3[inv_perm]              (host add + gather)
      out_con = out_in[perm]                  (host gather)

kernel(**inputs) takes the full unsharded inputs and returns [2, E, D] f32.
"""
import sys

if "/opt/trn_rl_repo" not in sys.path:
    sys.path.insert(0, "/opt/trn_rl_repo")

from contextlib import ExitStack

import ml_dtypes
import numpy as np

import concourse.mybir as mybir
import concourse.tile as tile
from concourse import bacc
from concourse.bass_utils import run_bass_kernel_spmd

F32 = mybir.dt.float32
BF16 = mybir.dt.bfloat16
AF = mybir.ActivationFunctionType
ALU = mybir.AluOpType
BF16_NP = ml_dtypes.bfloat16

N, DV, M, DE, E = 2048, 32, 4096, 16, 65536
D, WD, L, H = 256, 64, 2, 4
NCORES = 8
MASK_C = 30.0
SLAB = 512
R = E // NCORES
NSLAB = R // SLAB


def _sab_layer(nc, pools, X, C, l):
    """One SAB layer on a 512-row slab. X = [fb] of [128,512] bf16 (feat-major).
    C holds per-side consts. Returns Z tiles (same layout)."""
    sb, sbQKV, sbO, sbX, sbE, pmm, pS, pO, pRB = pools
    Wl = C["W"][l]          # Wl[p][kb] = [128,256] bf16 tiles, p in q,k,v,o
    Bcol = C["Bcol"][l]     # [128,4] f32: cols p*2+fb for p in (q,k)
    bvbc = C["bvbc"][l]     # [128,512] bf16 broadcast of bv (rb2-tiled)
    boS = C["boS"][l]       # [1,256] bf16
    Gk, Gq = C["Gk"], C["Gq"]
    ones64, onesrow, negc = C["ones64"], C["onesrow"], C["negc"]

    # Q (Act copy w/ bias), K (Pool copy w/ bias): feat-major [128,512]
    Qt, Kt = [], []
    for fb in range(2):
        psQ = pmm.tile([128, 512], F32, tag="mm", name="psQ")
        for kb in range(2):
            nc.tensor.matmul(psQ[:], Wl["q"][kb][:, fb * 128:(fb + 1) * 128],
                             X[kb][:], start=(kb == 0), stop=(kb == 1))
        qt = sbQKV.tile([128, 512], BF16, tag="Qt", name="Qt")
        nc.scalar.activation(qt[:], psQ[:], AF.Identity, bias=Bcol[:, fb:fb + 1])
        Qt.append(qt)
    for fb in range(2):
        psK = pmm.tile([128, 512], F32, tag="mm", name="psK")
        for kb in range(2):
            nc.tensor.matmul(psK[:], Wl["k"][kb][:, fb * 128:(fb + 1) * 128],
                             X[kb][:], start=(kb == 0), stop=(kb == 1))
        kt = sbQKV.tile([128, 512], BF16, tag="Kt", name="Kt")
        nc.gpsimd.tensor_scalar_add(kt[:], psK[:], Bcol[:, 2 + fb:3 + fb])
        Kt.append(kt)

    # V row-major: Vt[vv] packs rb=2vv (cols 0:256) and rb=2vv+1 (cols 256:512)
    Vt = []
    for vv in range(2):
        psV = pmm.tile([128, 512], F32, tag="mm", name="psV")
        for rb2 in range(2):
            rb = vv * 2 + rb2
            reg = psV[:, rb2 * 256:(rb2 + 1) * 256]
            for kb in range(2):
                nc.tensor.matmul(reg, X[kb][:, rb * 128:(rb + 1) * 128],
                                 Wl["v"][kb][:], start=(kb == 0), stop=(kb == 1),
                                 skip_group_check=True)
        vt = sbQKV.tile([128, 512], BF16, tag="Vt", name="Vt")
        nc.gpsimd.tensor_add(vt[:], psV[:], bvbc[:])
        Vt.append(vt)

    # scores (diag 128x128 blocks, 4 heads batched per PSUM bank) + exp
    eS = sbE.tile([128, 2048], BF16, tag="eS", name="eS")
    eSr = eS[:].rearrange("p (a b) -> p a b", a=4, b=512)
    for qb in range(4):
        qs = slice(qb * 128, (qb + 1) * 128)
        psS = pS.tile([128, 512], F32, tag="psS", name="psS")
        nc.tensor.matmul(psS[:], Gk[:], Gq[:], start=True, stop=False,
                         skip_group_check=True)
        for h in range(4):
            pr, off = h // 2, (h % 2) * 64
            nc.tensor.matmul(psS[:, h * 128:(h + 1) * 128],
                             Kt[pr][off:off + 64, qs], Qt[pr][off:off + 64, qs],
                             start=False, stop=True, skip_group_check=True)
        nc.scalar.activation(eS[:, qb * 512:(qb + 1) * 512], psS[:], AF.Exp,
                             bias=negc[:], scale=1.0 / 16.0)

    # denominators broadcast into O-layout [128f(2 heads), 512q] per fb
    psDEN = [pRB.tile([128, 512], F32, tag="psDEN", name="psDEN") for _ in range(2)]
    for h in range(4):
        fb, off = h // 2, (h % 2) * 64
        nc.tensor.matmul(psDEN[fb][off:off + 64, :],
                         ones64[:], eSr[:, :, h * 128:(h + 1) * 128],
                         start=True, stop=True, skip_group_check=True)

    # AV into O-layout psum [128f(2 heads), 512q] per fb
    psOf = [pO.tile([128, 512], F32, tag="psO", name="psO") for _ in range(2)]
    for qb in range(4):
        for h in range(4):
            fb, off = h // 2, (h % 2) * 64
            nc.tensor.matmul(
                psOf[fb][off:off + 64, qb * 128:(qb + 1) * 128],
                Vt[qb // 2][:, (qb % 2) * 256 + h * 64:(qb % 2) * 256 + h * 64 + 64],
                eS[:, qb * 512 + h * 128:qb * 512 + (h + 1) * 128],
                start=True, stop=True, skip_group_check=True)

    # O = psO / den + Q
    Ot = []
    for fb in range(2):
        U = sb.tile([128, 512], BF16, tag="Ut", name="Ut")
        nc.vector.tensor_tensor(U[:], psOf[fb][:], psDEN[fb][:], ALU.divide)
        O = sbO.tile([128, 512], BF16, tag="Ot", name="Ot")
        nc.vector.tensor_add(O[:], U[:], Qt[fb][:])
        Ot.append(O)

    # Z = O + relu(O @ Wo + bo)
    Zt = []
    for fb in range(2):
        psR = pRB.tile([128, 512], F32, tag="psDEN", name="psR")
        for kb in range(2):
            nc.tensor.matmul(psR[:], Wl["o"][kb][:, fb * 128:(fb + 1) * 128],
                             Ot[kb][:], start=(kb == 0), stop=False,
                             skip_group_check=True)
        nc.tensor.matmul(psR[:], boS[0:1, fb * 128:(fb + 1) * 128], onesrow[:],
                         start=False, stop=True, skip_group_check=True)
        Z = sbX.tile([128, 512], BF16, tag="Xt", name="Zt")
        nc.vector.scalar_tensor_tensor(Z[:], psR[:], 0.0, Ot[fb][:],
                                       ALU.max, ALU.add)
        Zt.append(Z)
    return Zt


def _load_side_consts(nc, const, tag, dram, G):
    """One batched DMA per constant group; expose AP slices."""
    C = {}
    Wall = const.tile([128, 16 * 256], BF16, tag=f"{tag}Wall", name=f"{tag}Wall")
    nc.sync.dma_start(
        Wall[:].rearrange("p (i d) -> p i d", i=16, d=256),
        dram["W"].rearrange("l pi (kb p) d -> p (l pi kb) d", kb=2, p=128))
    C["W"] = []
    for l in range(L):
        Wl = {}
        for pi, p in enumerate(["q", "k", "v", "o"]):
            Wl[p] = []
            for kb in range(2):
                i = (l * 4 + pi) * 2 + kb
                Wl[p].append(Wall[:, i * 256:(i + 1) * 256])
        C["W"].append(Wl)
    Ball = const.tile([128, L * 4], F32, tag=f"{tag}Ball", name=f"{tag}Ball")
    nc.sync.dma_start(Ball[:].rearrange("p (l c) -> p l c", l=L, c=4),
                      dram["Bcol"].rearrange("l p c -> p l c"))
    C["Bcol"] = [Ball[:, l * 4:(l + 1) * 4] for l in range(L)]
    bvall = const.tile([128, L * 512], BF16, tag=f"{tag}bvall", name=f"{tag}bvall")
    nc.sync.dma_start(bvall[:].rearrange("p (l c) -> p l c", l=L, c=512),
                      dram["bvbc"].rearrange("l p c -> p l c"))
    C["bvbc"] = [bvall[:, l * 512:(l + 1) * 512] for l in range(L)]
    boall = const.tile([1, L * 256], BF16, tag=f"{tag}boall", name=f"{tag}boall")
    nc.sync.dma_start(boall[:].rearrange("o (l c) -> o l c", l=L, c=256),
                      dram["boS"].rearrange("l o c -> o l c"))
    C["boS"] = [boall[:, l * 256:(l + 1) * 256] for l in range(L)]
    gk = const.tile([G, 128], BF16, tag=f"{tag}Gk", name=f"{tag}Gk")
    nc.sync.dma_start(gk[:], dram["Gk"])
    C["Gk"] = gk
    gq = const.tile([G, 512], BF16, tag=f"{tag}Gq", name=f"{tag}Gq")
    nc.sync.dma_start(gq[:], dram["Gq"])
    C["Gq"] = gq
    return C


def build_program():
    nc = bacc.Bacc("TRN2", target_bir_lowering=False, debug=False)

    dram = {}

    def din(name, shape, dt=BF16):
        dram[name] = nc.dram_tensor(name, shape, dt, kind="ExternalInput").ap()
        return dram[name]

    xv_d = din("xv", [2, 128, R])
    wv_d = din("wv", [WD, R])
    xe_d = din("xe", [2, 128, R])
    we_d = din("we", [WD, R])
    x0_d = din("x0", [2, 128, R])
    peW_v_d = din("peW_v", [WD, D])
    peW_e_d = din("peW_e", [WD, D])
    peb_d = din("peb", [128, 4], F32)  # cols: side*2+fb
    W_v_d = din("W_v", [L, 4, D, D])
    W_e_d = din("W_e", [L, 4, D, D])
    Bcol_v_d = din("Bcol_v", [L, 128, 4], F32)
    Bcol_e_d = din("Bcol_e", [L, 128, 4], F32)
    bvbc_v_d = din("bvbc_v", [L, 128, 512])
    bvbc_e_d = din("bvbc_e", [L, 128, 512])
    boS_v_d = din("boS_v", [L, 1, D])
    boS_e_d = din("boS_e", [L, 1, D])
    Wupd_d = din("W_upd", [4, D, D])
    updb_d = din("updb_col", [128, 2], F32)
    Gk_v_d = din("Gk_v", [4, 128])
    Gq_v_d = din("Gq_v", [4, 512])
    Gk_e_d = din("Gk_e", [8, 128])
    Gq_e_d = din("Gq_e", [8, 512])
    ones64_d = din("ones64", [128, 64])
    onesrow_d = din("onesrow", [1, 512])

    A_d = nc.dram_tensor("A", [D, R], F32, kind="ExternalOutput").ap()
    P3_d = nc.dram_tensor("P3", [D, R], F32, kind="ExternalOutput").ap()

    with tile.TileContext(nc) as tc, ExitStack() as es, \
            nc.allow_low_precision(reason="bf16 pipeline, fp32 accum in PSUM"):
        const = es.enter_context(tc.tile_pool(name="const", bufs=1))
        resid = es.enter_context(tc.tile_pool(name="resid", bufs=1))
        sb = es.enter_context(tc.tile_pool(name="sb", bufs=3))
        sbQKV = es.enter_context(tc.tile_pool(name="sbQKV", bufs=3))
        sbO = es.enter_context(tc.tile_pool(name="sbO", bufs=4))
        sbX = es.enter_context(tc.tile_pool(name="sbX", bufs=4))
        sbE = es.enter_context(tc.tile_pool(name="sbE", bufs=4))
        outp = es.enter_context(tc.tile_pool(name="outp", bufs=2))
        pmm = es.enter_context(tc.tile_pool(name="pmm", bufs=2, space="PSUM"))
        pS = es.enter_context(tc.tile_pool(name="pS", bufs=2, space="PSUM"))
        pO = es.enter_context(tc.tile_pool(name="pO", bufs=2, space="PSUM"))
        pRB = es.enter_context(tc.tile_pool(name="pRB", bufs=2, space="PSUM"))
        pools = (sb, sbQKV, sbO, sbX, sbE, pmm, pS, pO, pRB)

        # consts
        negc = const.tile([128, 1], F32, tag="negc", name="negc")
        nc.vector.memset(negc[:], -MASK_C)
        ones64 = const.tile([128, 64], BF16, tag="ones64", name="ones64")
        nc.sync.dma_start(ones64[:], ones64_d)
        onesrow = const.tile([1, 512], BF16, tag="onesrow", name="onesrow")
        nc.sync.dma_start(onesrow[:], onesrow_d)
        peW, peb = {}, {}
        pebt = const.tile([128, 4], F32, tag="peb", name="peb")
        nc.sync.dma_start(pebt[:], peb_d)
        for si, (s, peW_d) in enumerate((("v", peW_v_d), ("e", peW_e_d))):
            t = const.tile([WD, D], BF16, tag=f"peW_{s}", name=f"peW_{s}")
            nc.sync.dma_start(t[:], peW_d)
            peW[s] = t
            peb[s] = pebt[:, si * 2:si * 2 + 2]

        side_consts = {}
        for s, W_d, Bc_d, bv_d, bo_d, Gk_d, Gq_d, G in (
                ("v", W_v_d, Bcol_v_d, bvbc_v_d, boS_v_d, Gk_v_d, Gq_v_d, 4),
                ("e", W_e_d, Bcol_e_d, bvbc_e_d, boS_e_d, Gk_e_d, Gq_e_d, 8)):
            side_consts[s] = _load_side_consts(
                nc, const, s,
                {"W": W_d, "Bcol": Bc_d, "bvbc": bv_d, "boS": bo_d,
                 "Gk": Gk_d, "Gq": Gq_d}, G)
            side_consts[s]["ones64"] = ones64
            side_consts[s]["onesrow"] = onesrow
            side_consts[s]["negc"] = negc

        Wu_all = const.tile([128, 8 * 256], BF16, tag="WuAll", name="WuAll")
        nc.sync.dma_start(
            Wu_all[:].rearrange("p (i d) -> p i d", i=8, d=256),
            Wupd_d.rearrange("j (kb p) d -> p (j kb) d", kb=2, p=128))
        Wupd = [[Wu_all[:, (j * 2 + kb) * 256:(j * 2 + kb + 1) * 256]
                 for kb in range(2)] for j in range(4)]
        updbcol = const.tile([128, 2], F32, tag="updb", name="updb")
        nc.sync.dma_start(updbcol[:], updb_d)

        # resident inputs (order: v first so slab 0 can start early)
        xres, wres = {}, {}
        for s, x_d, w_d in (("v", xv_d, wv_d), ("e", xe_d, we_d)):
            xres[s] = []
            for fb in range(2):
                t = resid.tile([128, R], BF16, tag=f"x{s}{fb}", name=f"x{s}{fb}")
                nc.sync.dma_start(t[:], x_d[fb])
                xres[s].append(t)
            wt = resid.tile([WD, R], BF16, tag=f"w{s}", name=f"w{s}")
            nc.sync.dma_start(wt[:], w_d)
            wres[s] = wt
        x0res = []
        for fb in range(2):
            t = resid.tile([128, R], BF16, tag=f"x0{fb}", name=f"x0{fb}")
            nc.sync.dma_start(t[:], x0_d[fb])
            x0res.append(t)


        for t in range(NSLAB):
            for side in ("v", "e"):
                C = side_consts[side]
                cs = slice(t * SLAB, (t + 1) * SLAB)
                # mailbox: X = x + peW^T w + peb
                X = []
                for fb in range(2):
                    psP = pmm.tile([128, 512], F32, tag="mm", name="psP")
                    nc.tensor.matmul(psP[:], peW[side][:, fb * 128:(fb + 1) * 128],
                                     wres[side][:, cs], start=True, stop=True)
                    xt = sbX.tile([128, 512], BF16, tag="Xt", name="Xt")
                    nc.vector.scalar_tensor_tensor(
                        xt[:], psP[:], peb[side][:, fb:fb + 1],
                        xres[side][fb][:, cs], ALU.add, ALU.add)
                    X.append(xt)

                for l in range(L):
                    X = _sab_layer(nc, pools, X, C, l)

                if side == "v":
                    # A^T[fb] = sum_j Wupd_j[:,fb]^T @ src_j + updb (feat-major)
                    Ao = outp.tile([128, 1024], F32, tag="Out", name="Aout")
                    for fb in range(2):
                        psA = pO.tile([128, 512], F32, tag="psO", name="psA")
                        first = True
                        for srcs, j in ((None, 0), (X, 1), (None, 3)):
                            for kb in range(2):
                                if srcs is None:
                                    base = xres["v"] if j == 0 else x0res
                                    rhs = base[kb][:, cs]
                                else:
                                    rhs = srcs[kb][:]
                                nc.tensor.matmul(
                                    psA[:], Wupd[j][kb][:, fb * 128:(fb + 1) * 128],
                                    rhs, start=first, stop=(j == 3 and kb == 1),
                                    skip_group_check=True)
                                first = False
                        if fb == 0:
                            nc.vector.tensor_scalar_add(
                                Ao[:, 0:512], psA[:], updbcol[:, 0:1])
                        else:
                            nc.scalar.activation(
                                Ao[:, 512:1024], psA[:], AF.Identity,
                                bias=updbcol[:, 1:2])
                    nc.sync.dma_start(
                        A_d[:, cs].rearrange("(a p) r -> p a r", a=2, p=128),
                        Ao[:].rearrange("p (a r) -> p a r", a=2, r=512))
                else:
                    Po = outp.tile([128, 1024], F32, tag="Out", name="Pout")
                    for fb in range(2):
                        psP3 = pO.tile([128, 512], F32, tag="psO", name="psP3")
                        for kb in range(2):
                            nc.tensor.matmul(
                                psP3[:], Wupd[2][kb][:, fb * 128:(fb + 1) * 128],
                                X[kb][:], start=(kb == 0), stop=(kb == 1),
                                skip_group_check=True)
                        if fb == 0:
                            nc.vector.tensor_copy(Po[:, 0:512], psP3[:])
                        else:
                            nc.scalar.copy(Po[:, 512:1024], psP3[:])
                    nc.sync.dma_start(
                        P3_d[:, cs].rearrange("(a p) r -> p a r", a=2, p=128),
                        Po[:].rearrange("p (a r) -> p a r", a=2, r=512))

    nc.compile()
    return nc


def _make_group_consts(n_group):
    G = 128 // n_group
    Gk = np.zeros((G, 128), BF16_NP)
    for g in range(G):
        Gk[g, g * n_group:(g + 1) * n_group] = 16.0 * MASK_C
    Gq = np.zeros((G, 512), BF16_NP)
    for h in range(4):
        for g in range(G):
            q0 = h * 128 + g * n_group
            Gq[g, q0:q0 + n_group] = 1.0
    return Gk, Gq


_PROGRAM_CACHE = {}


def _get_program(_r=R):
    if _r not in _PROGRAM_CACHE:
        _PROGRAM_CACHE[_r] = build_program()
    return _PROGRAM_CACHE[_r]


def kernel(co_feat_in, co_feat_con, co_feat_0, weight_in, weight_con,
           pe_v_W, pe_v_b, pe_e_W, pe_e_b,
           Wq_v, bq_v, Wk_v, bk_v, Wv_v, bv_v, Wo_v, bo_v,
           Wq_e, bq_e, Wk_e, bk_e, Wv_e, bv_e, Wo_e, bo_e,
           upd_W, upd_b, perm):
    f32 = lambda x: np.asarray(x, np.float32)
    bf = lambda x: np.asarray(x).astype(BF16_NP)
    perm = np.asarray(perm)

    nc = _get_program()

    Gk_v, Gq_v = _make_group_consts(DV)
    Gk_e, Gq_e = _make_group_consts(DE)

    def bcol(bq, bk):
        out = np.zeros((L, 128, 4), np.float32)
        for l in range(L):
            for pi, b in enumerate((f32(bq), f32(bk))):
                for fb in range(2):
                    out[l, :, pi * 2 + fb] = b[l, fb * 128:(fb + 1) * 128]
        return out

    peb = np.zeros((128, 4), np.float32)
    for si, b in enumerate((f32(pe_v_b), f32(pe_e_b))):
        for fb in range(2):
            peb[:, si * 2 + fb] = b[fb * 128:(fb + 1) * 128]

    shared = {
        "peW_v": bf(pe_v_W), "peW_e": bf(pe_e_W), "peb": peb,
        "W_v": np.stack([bf(Wq_v), bf(Wk_v), bf(Wv_v), bf(Wo_v)], axis=1).copy(),
        "W_e": np.stack([bf(Wq_e), bf(Wk_e), bf(Wv_e), bf(Wo_e)], axis=1).copy(),
        "Bcol_v": bcol(bq_v, bk_v), "Bcol_e": bcol(bq_e, bk_e),
        "bvbc_v": np.ascontiguousarray(np.broadcast_to(
            np.tile(bf(bv_v), (1, 2))[:, None, :], (L, 128, 512))),
        "bvbc_e": np.ascontiguousarray(np.broadcast_to(
            np.tile(bf(bv_e), (1, 2))[:, None, :], (L, 128, 512))),
        "boS_v": bf(bo_v).reshape(L, 1, D).copy(),
        "boS_e": bf(bo_e).reshape(L, 1, D).copy(),
        "W_upd": bf(upd_W).reshape(4, D, D).copy(),
        "updb_col": f32(upd_b).reshape(2, 128).T.copy(),
        "Gk_v": Gk_v, "Gq_v": Gq_v, "Gk_e": Gk_e, "Gq_e": Gq_e,
        "ones64": np.ones((128, 64), BF16_NP),
        "onesrow": np.ones((1, 512), BF16_NP),
    }

    def xsplit(a, rs):
        t = np.ascontiguousarray(np.asarray(a)[rs].T.astype(BF16_NP))
        return t.reshape(2, 128, R)

    in_maps = []
    for c in range(NCORES):
        rs = slice(c * R, (c + 1) * R)
        m = dict(shared)
        m["xv"] = xsplit(co_feat_in, rs)
        m["wv"] = np.ascontiguousarray(np.asarray(weight_in)[rs].T.astype(BF16_NP))
        m["xe"] = xsplit(co_feat_con, rs)
        m["we"] = np.ascontiguousarray(np.asarray(weight_con)[rs].T.astype(BF16_NP))
        m["x0"] = xsplit(co_feat_0, rs)
        in_maps.append(m)

    res = run_bass_kernel_spmd(nc, in_maps, core_ids=list(range(NCORES)))
    A = np.concatenate([res.results[c]["A"].T for c in range(NCORES)], axis=0)
    P3 = np.concatenate([res.results[c]["P3"].T for c in range(NCORES)], axis=0)

    inv_perm = np.argsort(perm)
    out_in = A + P3[inv_perm]
    return np.stack([out_in, out_in[perm]]).astype(np.float32)
